# revision 9
# baseline (speedup 1.0000x reference)
"""Trainium2 Bass kernel for nn_MoEAggregator.

Reference computation:
    pooled       = x[:, -1, :]                         # [B, D]
    gates        = pooled @ gate_W.T + gate_b          # [B, N]
    top2 idx     = top_k(gates, 2)                     # [B, 2]
    out          = base_res + sum_k lora[..., idx_k]   # [B, S, D]

Shapes (hardcoded): B=2, S=2048, D=4096, N=8, top_k=2, fp32.

Strategy (8 NeuronCores, data-parallel over B*S rows):
  Phase A (device, replicated): gate matmul + bias via one fused
      tensor_tensor_reduce on a [16=B*N, D] layout, stream-transpose to
      [B, N], then DVE max/max_index -> top-2 adapter indices.
  Host: slices the two selected adapter planes per batch out of
      lora_results (data movement only; adapter dim is innermost so the
      device could not read the selected planes contiguously anyway) and
      shards all big tensors by row across the 8 cores.
  Phase B (device, 8-way): streaming out = base + a0 + a1 with
      [128, 4096] tiles, multi-buffered so DMA stays saturated.
"""

import json

import numpy as np

import bass_rust
import concourse.bass as bass
import concourse.bass2jax as bass2jax
import concourse.mybir as mybir
from concourse.bass_utils import run_bass_kernel_spmd
from concourse.tile import TileContext


def _split_multi_waits(bir_bytes: bytes) -> bytes:
    """This container's walrus build allows only ONE sync-wait per
    instruction; Tile's kernel-tail drain carries one per DMA sem lane.
    Split extras onto duplicated predecessor instructions (same engine,
    one wait each) so codegen accepts the module."""
    m = json.loads(bir_bytes)
    changed = False
    for fn in m.get("functions", []):
        for bb in fn.get("blocks", []):
            new_insts = []
            for inst in bb.get("instructions", []):
                si = inst.get("sync_info") or {}
                ow = si.get("on_wait") or []
                if len(ow) > 1:
                    changed = True
                    for k, w in enumerate(ow[:-1]):
                        new_insts.append(
                            {
                                "name": f"{inst['name']}_w{k}",
                                "opcode": "Drain",
                                "engine": inst["engine"],
                                "ins": [],
                                "outs": [],
                                "debug": inst.get("debug"),
                                "sync_info": {"on_wait": [w]},
                            }
                        )
                    si["on_wait"] = [ow[-1]]
                    inst["sync_info"] = si
                new_insts.append(inst)
            bb["instructions"] = new_insts
    return json.dumps(m).encode() if changed else bir_bytes


if not getattr(bass2jax, "_moe_wait_patch", False):
    _orig_compile_bir = bass2jax.compile_bir_kernel

    def _compile_bir_patched(bir_json, tmpdir, neff_name="file.neff"):
        return _orig_compile_bir(
            _split_multi_waits(bir_json), tmpdir, neff_name=neff_name
        )

    bass2jax.compile_bir_kernel = _compile_bir_patched
    bass2jax._moe_wait_patch = True

B, S, D, N, TOPK = 2, 2048, 4096, 8, 2
NCORES = 8
ROWS = B * S            # 4096 token rows
RPC = ROWS // NCORES    # 512 rows per core
F32 = mybir.dt.float32
U32 = mybir.dt.uint32

# set by test harness to collect profiling info
PROFILE = False
TRACE_CORES = [0]
LAST_EXEC_NS = {}

_cache = {}


def _build_router() -> bass.Bass:
    """gates[b,n] = gate_b[n] + sum_d pooled[b,d] * gate_W[n,d]; top-2 idx.

    Inputs (replicated on every core):
      p16 [16, D]  row r = pooled[r // 8]
      w16 [16, D]  row r = gate_W[r % 8]
      b16 [16, 1]  row r = gate_b[r % 8]
    Output: idx [2, 8] int32, first TOPK entries per row are the selection.
    """
    nc = bass.Bass()
    p16 = nc.declare_dram_parameter("p16", [16, D], F32, isOutput=False)
    w16 = nc.declare_dram_parameter("w16", [16, D], F32, isOutput=False)
    b16 = nc.declare_dram_parameter("b16", [16, 1], F32, isOutput=False)
    idx = nc.declare_dram_parameter("idx", [1, 16], U32, isOutput=True)

    with TileContext(nc) as tc:
        with tc.tile_pool(name="sbuf", bufs=1) as pool:
            tp = pool.tile([16, D], F32)
            tw = pool.tile([16, D], F32)
            tb = pool.tile([16, 1], F32)
            nc.sync.dma_start(out=tp, in_=p16[:, :])
            nc.sync.dma_start(out=tw, in_=w16[:, :])
            nc.sync.dma_start(out=tb, in_=b16[:, :])

            # g16[r] = b16[r] + sum_d p16[r,d]*w16[r,d]
            # (tensor_tensor_reduce lowers to an ISA op this walrus build
            # rejects, so use mul + reduce + add instead)
            prod = pool.tile([16, D], F32)
            g16 = pool.tile([16, 1], F32)
            nc.vector.tensor_mul(out=prod, in0=tp, in1=tw)
            nc.vector.reduce_sum(out=g16, in_=prod, axis=bass_rust.AxisListType.X)
            nc.vector.tensor_add(out=g16, in0=g16, in1=tb)

            # Move the 16 per-partition values into partition 0's free dim:
            # t32[r, 0] = g16[r]; transpose -> tt[0, r] = g16[r]
            t32 = pool.tile([32, 32], F32)
            nc.vector.memset(t32, 0.0)
            nc.vector.tensor_copy(out=t32[0:16, 0:1], in_=g16)
            tt = pool.tile([32, 32], F32)
            nc.vector.transpose(out=tt, in_=t32)

            mx = pool.tile([1, 16], F32)
            ix = pool.tile([1, 16], U32)
            for b in range(2):
                gates_b = tt[0:1, 8 * b : 8 * b + 8]
                nc.vector.max(out=mx[0:1, 8 * b : 8 * b + 8], in_=gates_b)
                nc.vector.max_index(
                    out=ix[0:1, 8 * b : 8 * b + 8],
                    in_max=mx[0:1, 8 * b : 8 * b + 8],
                    in_values=gates_b,
                )
            nc.sync.dma_start(out=idx[:, :], in_=ix)
    return nc


def _build_adder() -> bass.Bass:
    """out = base + a0 + a1, streaming [RPC, D] per core."""
    nc = bass.Bass()
    base = nc.declare_dram_parameter("base", [RPC, D], F32, isOutput=False)
    a0 = nc.declare_dram_parameter("a0", [RPC, D], F32, isOutput=False)
    a1 = nc.declare_dram_parameter("a1", [RPC, D], F32, isOutput=False)
    out = nc.declare_dram_parameter("out", [RPC, D], F32, isOutput=True)

    P = 128
    ntiles = RPC // P
    with TileContext(nc) as tc:
        with tc.tile_pool(name="sbuf", bufs=3) as pool:
            for i in range(ntiles):
                rows = slice(i * P, (i + 1) * P)
                tb = pool.tile([P, D], F32)
                t0 = pool.tile([P, D], F32)
                t1 = pool.tile([P, D], F32)
                nc.sync.dma_start(out=tb, in_=base[rows])
                nc.sync.dma_start(out=t0, in_=a0[rows])
                nc.sync.dma_start(out=t1, in_=a1[rows])
                nc.vector.tensor_add(out=t0, in0=t0, in1=tb)
                nc.vector.tensor_add(out=t0, in0=t0, in1=t1)
                nc.sync.dma_start(out=out[rows], in_=t0)
    return nc


def _run(tag: str, build, in_maps):
    if tag not in _cache:
        _cache[tag] = build()
    nc = _cache[tag]
    res = run_bass_kernel_spmd(
        nc,
        in_maps,
        list(range(NCORES)),
        trace=PROFILE,
        trace_cores=TRACE_CORES if PROFILE else None,
    )
    if PROFILE:
        LAST_EXEC_NS[tag] = res.exec_time_ns
    return res.results


def kernel(x, base_res, lora_results, gate_W, gate_b, top_k):
    assert int(top_k) == TOPK
    x = np.asarray(x, dtype=np.float32)
    base_res = np.asarray(base_res, dtype=np.float32)
    lora_results = np.asarray(lora_results, dtype=np.float32)
    gate_W = np.asarray(gate_W, dtype=np.float32)
    gate_b = np.asarray(gate_b, dtype=np.float32)

    # ---- Phase A: routing on device (replicated on all cores) ----
    pooled = x[:, -1, :]                                   # [B, D]
    p16 = np.ascontiguousarray(np.repeat(pooled, N, axis=0))
    w16 = np.ascontiguousarray(np.tile(gate_W, (B, 1)))
    b16 = np.ascontiguousarray(np.tile(gate_b, B).reshape(B * N, 1))
    a_in = [{"p16": p16, "w16": w16, "b16": b16} for _ in range(NCORES)]
    a_res = _run("router", _build_router, a_in)
    idx = np.asarray(a_res[0]["idx"]).reshape(B, N)       # [2, 8] uint32
    sel = idx[:, :TOPK].astype(np.int64)                   # [B, TOPK]

    # ---- Host: shard + gather selected adapter planes ----
    base_flat = base_res.reshape(ROWS, D)
    b_in = []
    spb = S // (NCORES // B)                               # seq rows per core
    for c in range(NCORES):
        r0 = c * RPC
        b = r0 // S
        s0 = r0 - b * S
        shard = {
            "base": np.ascontiguousarray(base_flat[r0 : r0 + RPC]),
            "a0": np.ascontiguousarray(
                lora_results[b, s0 : s0 + RPC, :, sel[b, 0]]
            ),
            "a1": np.ascontiguousarray(
                lora_results[b, s0 : s0 + RPC, :, sel[b, 1]]
            ),
        }
        b_in.append(shard)

    # ---- Phase B: streaming aggregation on 8 cores ----
    b_res = _run("adder", _build_adder, b_in)
    out = np.concatenate([np.asarray(b_res[c]["out"]) for c in range(NCORES)])
    return out.reshape(B, S, D)


# revision 12
# speedup vs baseline: 1.2929x; 1.2929x over previous
"""Trainium2 Bass kernel for nn_MoEAggregator.

Reference computation:
    pooled       = x[:, -1, :]                         # [B, D]
    gates        = pooled @ gate_W.T + gate_b          # [B, N]
    top2 idx     = top_k(gates, 2)                     # [B, 2]
    out          = base_res + sum_k lora[..., idx_k]   # [B, S, D]

Shapes (hardcoded): B=2, S=2048, D=4096, N=8, top_k=2, fp32.

Strategy (8 NeuronCores, data-parallel over B*S rows):
  Phase A (device, replicated): gate matmul + bias via one fused
      tensor_tensor_reduce on a [16=B*N, D] layout, stream-transpose to
      [B, N], then DVE max/max_index -> top-2 adapter indices.
  Host: slices the two selected adapter planes per batch out of
      lora_results (data movement only; adapter dim is innermost so the
      device could not read the selected planes contiguously anyway) and
      shards all big tensors by row across the 8 cores.
  Phase B (device, 8-way): streaming out = base + a0 + a1 with
      [128, 4096] tiles, multi-buffered so DMA stays saturated.
"""

import json

import numpy as np

import bass_rust
import concourse.bass as bass
import concourse.bass2jax as bass2jax
import concourse.mybir as mybir
from concourse.bass_utils import run_bass_kernel_spmd
from concourse.tile import TileContext


def _split_multi_waits(bir_bytes: bytes) -> bytes:
    """This container's walrus build allows only ONE sync-wait per
    instruction; Tile's kernel-tail drain carries one per DMA sem lane.
    Split extras onto duplicated predecessor instructions (same engine,
    one wait each) so codegen accepts the module."""
    m = json.loads(bir_bytes)
    changed = False
    for fn in m.get("functions", []):
        for bb in fn.get("blocks", []):
            new_insts = []
            for inst in bb.get("instructions", []):
                si = inst.get("sync_info") or {}
                ow = si.get("on_wait") or []
                if len(ow) > 1:
                    changed = True
                    for k, w in enumerate(ow[:-1]):
                        new_insts.append(
                            {
                                "name": f"{inst['name']}_w{k}",
                                "opcode": "Drain",
                                "engine": inst["engine"],
                                "ins": [],
                                "outs": [],
                                "debug": inst.get("debug"),
                                "sync_info": {"on_wait": [w]},
                            }
                        )
                    si["on_wait"] = [ow[-1]]
                    inst["sync_info"] = si
                new_insts.append(inst)
            bb["instructions"] = new_insts
    return json.dumps(m).encode() if changed else bir_bytes


if not getattr(bass2jax, "_moe_wait_patch", False):
    _orig_compile_bir = bass2jax.compile_bir_kernel

    def _compile_bir_patched(bir_json, tmpdir, neff_name="file.neff"):
        return _orig_compile_bir(
            _split_multi_waits(bir_json), tmpdir, neff_name=neff_name
        )

    bass2jax.compile_bir_kernel = _compile_bir_patched
    bass2jax._moe_wait_patch = True

B, S, D, N, TOPK = 2, 2048, 4096, 8, 2
NCORES = 8
ROWS = B * S            # 4096 token rows
RPC = ROWS // NCORES    # 512 rows per core
F32 = mybir.dt.float32
U32 = mybir.dt.uint32

# set by test harness to collect profiling info
PROFILE = False
TRACE_CORES = [0]
LAST_EXEC_NS = {}

_cache = {}


DC = D // 8  # 512: d-chunk per partition row in the router layout


def _build_router() -> bass.Bass:
    """gates[b,n] = gate_b[n] + sum_d pooled[b,d] * gate_W[n,d]; top-2 idx.

    128-partition layout: row r = g*8 + dc with g = b*8+n encodes chunk dc
    of gate g's dot product. DVE mul+reduce gives partials [128,1]; one PE
    matmul against a selector S (S[r,g]=1 iff r//8==g) collapses them to
    gates [1,16] in partition 0, where DVE max/max_index picks top-2.

    Inputs (replicated on every core):
      p128 [128, DC]  row r: pooled[b, dc*DC:(dc+1)*DC]
      w128 [128, DC]  row r: gate_W[n, dc*DC:(dc+1)*DC]
      s16  [128, 16]  np.repeat(eye(16), 8, axis=0)
      b16r [1, 16]    gate_b tiled per g
    Output: idx [1, 16] uint32; entries 8b..8b+1 are batch b's selection.
    """
    nc = bass.Bass()
    p128 = nc.declare_dram_parameter("p128", [128, DC], F32, isOutput=False)
    w128 = nc.declare_dram_parameter("w128", [128, DC], F32, isOutput=False)
    s16 = nc.declare_dram_parameter("s16", [128, 16], F32, isOutput=False)
    b16r = nc.declare_dram_parameter("b16r", [1, 16], F32, isOutput=False)
    idx = nc.declare_dram_parameter("idx", [1, 16], U32, isOutput=True)

    with TileContext(nc) as tc:
        with (
            tc.tile_pool(name="sbuf", bufs=1) as pool,
            tc.tile_pool(name="psum", bufs=1, space="PSUM") as psum_pool,
        ):
            tp = pool.tile([128, DC], F32)
            tw = pool.tile([128, DC], F32)
            ts = pool.tile([128, 16], F32)
            tb = pool.tile([1, 16], F32)
            nc.sync.dma_start(out=tp, in_=p128[:, :])
            nc.sync.dma_start(out=tw, in_=w128[:, :])
            nc.sync.dma_start(out=ts, in_=s16[:, :])
            nc.sync.dma_start(out=tb, in_=b16r[:, :])

            prod = pool.tile([128, DC], F32)
            part = pool.tile([128, 1], F32)
            nc.vector.tensor_mul(out=prod, in0=tp, in1=tw)
            nc.vector.reduce_sum(out=part, in_=prod, axis=bass_rust.AxisListType.X)

            pg = psum_pool.tile([1, 16], F32)
            nc.tensor.matmul(pg, part, ts, start=True, stop=True)

            gates = pool.tile([1, 16], F32)
            nc.vector.tensor_copy(out=gates, in_=pg)
            nc.vector.tensor_add(out=gates, in0=gates, in1=tb)

            mx = pool.tile([1, 16], F32)
            ix = pool.tile([1, 16], U32)
            for b in range(2):
                gates_b = gates[0:1, 8 * b : 8 * b + 8]
                nc.vector.max(out=mx[0:1, 8 * b : 8 * b + 8], in_=gates_b)
                nc.vector.max_index(
                    out=ix[0:1, 8 * b : 8 * b + 8],
                    in_max=mx[0:1, 8 * b : 8 * b + 8],
                    in_values=gates_b,
                )
            nc.sync.dma_start(out=idx[:, :], in_=ix)
    return nc


ADDER_COLS = 4096   # free-dim per tile
ADDER_BUFS = 4


def _build_adder() -> bass.Bass:
    """out = base + a0 + a1, streaming [RPC, D] per core.

    Loads issue on the SP HWDGE ring (nc.sync), stores on the Activation
    HWDGE ring (nc.scalar) so store waits never head-of-line-block loads.
    """
    nc = bass.Bass()
    base = nc.declare_dram_parameter("base", [RPC, D], F32, isOutput=False)
    a0 = nc.declare_dram_parameter("a0", [RPC, D], F32, isOutput=False)
    a1 = nc.declare_dram_parameter("a1", [RPC, D], F32, isOutput=False)
    out = nc.declare_dram_parameter("out", [RPC, D], F32, isOutput=True)

    P = 128
    cols = ADDER_COLS
    rows_total = RPC * D // cols
    ntiles = rows_total // P
    bviews = [t.rearrange("r (q c) -> (r q) c", c=cols) for t in (base, a0, a1)]
    oview = out.rearrange("r (q c) -> (r q) c", c=cols)
    with TileContext(nc) as tc:
        with tc.tile_pool(name="sbuf", bufs=ADDER_BUFS) as pool:
            for i in range(ntiles):
                rows = slice(i * P, (i + 1) * P)
                tb = pool.tile([P, cols], F32)
                t0 = pool.tile([P, cols], F32)
                t1 = pool.tile([P, cols], F32)
                nc.sync.dma_start(out=tb, in_=bviews[0][rows])
                nc.sync.dma_start(out=t0, in_=bviews[1][rows])
                nc.sync.dma_start(out=t1, in_=bviews[2][rows])
                nc.vector.tensor_add(out=t0, in0=t0, in1=tb)
                nc.vector.tensor_add(out=t0, in0=t0, in1=t1)
                nc.scalar.dma_start(out=oview[rows], in_=t0)
    return nc


def _run(tag: str, build, in_maps):
    if tag not in _cache:
        _cache[tag] = build()
    nc = _cache[tag]
    res = run_bass_kernel_spmd(
        nc,
        in_maps,
        list(range(NCORES)),
        trace=PROFILE,
        trace_cores=TRACE_CORES if PROFILE else None,
    )
    if PROFILE:
        LAST_EXEC_NS[tag] = res.exec_time_ns
    return res.results


def kernel(x, base_res, lora_results, gate_W, gate_b, top_k):
    assert int(top_k) == TOPK
    x = np.asarray(x, dtype=np.float32)
    base_res = np.asarray(base_res, dtype=np.float32)
    lora_results = np.asarray(lora_results, dtype=np.float32)
    gate_W = np.asarray(gate_W, dtype=np.float32)
    gate_b = np.asarray(gate_b, dtype=np.float32)

    # ---- Phase A: routing on device (replicated on all cores) ----
    pooled = x[:, -1, :]                                   # [B, D]
    # row r = (b*8+n)*8 + dc
    p128 = np.ascontiguousarray(
        np.broadcast_to(
            pooled.reshape(B, 1, 8, DC), (B, N, 8, DC)
        ).reshape(128, DC)
    )
    w128 = np.ascontiguousarray(
        np.broadcast_to(
            gate_W.reshape(1, N, 8, DC), (B, N, 8, DC)
        ).reshape(128, DC)
    )
    s16 = np.ascontiguousarray(np.repeat(np.eye(16, dtype=np.float32), 8, axis=0))
    b16r = np.ascontiguousarray(np.tile(gate_b, B).reshape(1, B * N))
    a_in = [
        {"p128": p128, "w128": w128, "s16": s16, "b16r": b16r}
        for _ in range(NCORES)
    ]
    a_res = _run("router", _build_router, a_in)
    idx = np.asarray(a_res[0]["idx"]).reshape(B, N)       # [2, 8] uint32
    sel = idx[:, :TOPK].astype(np.int64)                   # [B, TOPK]

    # ---- Host: shard + gather selected adapter planes ----
    base_flat = base_res.reshape(ROWS, D)
    b_in = []
    spb = S // (NCORES // B)                               # seq rows per core
    for c in range(NCORES):
        r0 = c * RPC
        b = r0 // S
        s0 = r0 - b * S
        shard = {
            "base": np.ascontiguousarray(base_flat[r0 : r0 + RPC]),
            "a0": np.ascontiguousarray(
                lora_results[b, s0 : s0 + RPC, :, sel[b, 0]]
            ),
            "a1": np.ascontiguousarray(
                lora_results[b, s0 : s0 + RPC, :, sel[b, 1]]
            ),
        }
        b_in.append(shard)

    # ---- Phase B: streaming aggregation on 8 cores ----
    b_res = _run("adder", _build_adder, b_in)
    out = np.concatenate([np.asarray(b_res[c]["out"]) for c in range(NCORES)])
    return out.reshape(B, S, D)
